# revision 13
# baseline (speedup 1.0000x reference)
"""Trainium2 Bass kernel for nn_Custom_U_2_88630945120527.

Dense transformer block:
    z = x @ W1.T                    # [L, B, P]
    kv = z @ W2.T ; k, v = split    # [L, B, 2P]
    scores = (z*scale) @ k.T        # per-batch [L, L], causal
    attn = softmax(scores)
    out = attn @ v                  # [L, B, P]
    y = (out + z) @ W3.T            # [L, B, D_OUT]

Sharding: data-parallel over batch (B=8 -> 8 cores, one batch element per
core, weights replicated, no collectives).

Per-core layout strategy: the host pre-transposes x (xT = x[:,b,:].T) and
the weights (W1T/W2T/W3T), so every matmul in the chain has its
contraction dim on partitions with zero on-chip transposes:
    zT[p,l]   = sum_d W1T[d,p] * xT[d,l]
    kT[q,l]   = sum_p W2T[p,q] * zT[p,l]          (q in [0,P))
    v[m,q]    = sum_p zT[p,m]  * W2T[p,P+q]
    sT[m,l]   = sum_p kT[p,m]  * zT[p,l]          (computed transposed!)
    eT[m,l]   = exp(scale*sT) with causal mask    (ScalarE, scale fused)
    sums[1,l] = sum_m ones[m] * eT[m,l]           (matmul w/ ones lhsT)
    oT[p,l]   = sum_m v[m,p]  * eT[m,l]
    resT[p,l] = oT[p,l] / sums[l] + zT[p,l]
    y[l,o]    = sum_p resT[p,l] * W3T[p,o]
Computing scores TRANSPOSED puts m (the softmax-reduction axis) on
partitions so attention weights feed the out-matmul as lhsT directly.
Softmax skips the max-subtraction (scores are O(5), exp is safe in fp32;
the reference's masked entries underflow to exactly 0, matched by zeroing).

All matmuls run as float32r (full PE rate, ~1.5e-4 rel err measured on HW
vs fp32) with fp32 PSUM accumulation.
"""

import numpy as np

L, B, D, P, DO = 2048, 8, 1024, 512, 1024
LC = 512  # l-chunk width (PSUM bank / fp32)
NLC = L // LC  # 4 chunks
NPT = P // 128  # 4 p-tiles
NDT = D // 128  # 8 d-tiles
NMT = L // 128  # 16 m-tiles
SCALE = float(P) ** -0.5

_CACHE = {}


def _build():
    import concourse.bacc as bacc
    import concourse.mybir as mybir
    import concourse.tile as tile
    from concourse.masks import make_upper_triangular

    F32 = mybir.dt.float32
    F32R = mybir.dt.float32r
    BF16 = mybir.dt.bfloat16

    def r(ap):
        return ap.bitcast(F32R)

    nc = bacc.Bacc("TRN2", debug=False)
    xT = nc.dram_tensor("xT", [D, L], F32R, kind="ExternalInput")
    w1t = nc.dram_tensor("w1t", [D, P], F32R, kind="ExternalInput")
    w2t = nc.dram_tensor("w2t", [P, 2 * P], F32R, kind="ExternalInput")
    w3t = nc.dram_tensor("w3t", [P, DO], F32R, kind="ExternalInput")
    y = nc.dram_tensor("y", [L, DO], F32, kind="ExternalOutput")

    with tile.TileContext(nc) as tc:
        with (
            tc.tile_pool(name="consts", bufs=1) as consts,
            tc.tile_pool(name="weights", bufs=1) as wpool,
            tc.tile_pool(name="persist", bufs=1) as persist,
            tc.tile_pool(name="work", bufs=2) as work,
            tc.tile_pool(name="etp", bufs=6) as etp,
            tc.tile_pool(name="psS", bufs=2, space="PSUM") as psS,
            tc.tile_pool(name="psO", bufs=1, space="PSUM") as psO,
            tc.tile_pool(name="psSum", bufs=2, space="PSUM") as psSum,
        ):
            # ---- constants ----
            zcol = consts.tile([128, 1], F32, tag="zcol")
            nc.vector.memset(zcol, 0.0)
            ones = consts.tile([128, 1], BF16, tag="ones")
            nc.vector.tensor_scalar_add(ones, zcol, 1.0)
            # tri[m, l] = 1.0 where l >= m else 0 (within a 128x128 block)
            tri = consts.tile([128, 128], BF16, tag="tri")
            make_upper_triangular(nc, tri, val=1.0, diag=True)

            # ---- weights (pre-transposed on host) ----
            w1 = [wpool.tile([128, P], F32, tag=f"w1_{d}", name=f"w1_{d}") for d in range(NDT)]
            for d in range(NDT):
                nc.sync.dma_start(r(w1[d]), w1t[d * 128 : (d + 1) * 128, :])
            w2 = [wpool.tile([128, 2 * P], F32, tag=f"w2_{p}", name=f"w2_{p}") for p in range(NPT)]
            for p in range(NPT):
                nc.sync.dma_start(r(w2[p]), w2t[p * 128 : (p + 1) * 128, :])
            w3 = [wpool.tile([128, DO], F32, tag=f"w3_{p}", name=f"w3_{p}") for p in range(NPT)]
            for p in range(NPT):
                nc.sync.dma_start(r(w3[p]), w3t[p * 128 : (p + 1) * 128, :])

            zt = [persist.tile([128, L], F32, tag=f"zt_{p}", name=f"zt_{p}") for p in range(NPT)]
            ztb = [persist.tile([128, L], BF16, tag=f"ztb_{p}", name=f"ztb_{p}") for p in range(NPT)]
            kt = [persist.tile([128, L], BF16, tag=f"kt_{p}", name=f"kt_{p}") for p in range(NPT)]
            vt = [persist.tile([128, P], BF16, tag=f"v_{t}", name=f"v_{t}") for t in range(NMT)]
            w2b = [persist.tile([128, 2 * P], BF16, tag=f"w2b_{p}", name=f"w2b_{p}") for p in range(NPT)]
            for p in range(NPT):
                nc.vector.tensor_copy(w2b[p], w2[p])

            # ---- phase 1: zT[p, l] ----
            for c in range(NLC):
                xtile = [work.tile([128, LC], F32, tag=f"xt_{d}", name=f"xt_{d}") for d in range(NDT)]
                for d in range(NDT):
                    nc.sync.dma_start(
                        r(xtile[d]),
                        xT[d * 128 : (d + 1) * 128, c * LC : (c + 1) * LC],
                    )
                for p in range(NPT):
                    ps = psS.tile([128, LC], F32, tag="psS")
                    for d in range(NDT):
                        nc.tensor.matmul(
                            ps,
                            r(w1[d][:, p * 128 : (p + 1) * 128]),
                            r(xtile[d]),
                            start=(d == 0),
                            stop=(d == NDT - 1),
                        )
                    nc.vector.tensor_copy(r(zt[p][:, c * LC : (c + 1) * LC]), ps)
                    nc.vector.tensor_copy(ztb[p][:, c * LC : (c + 1) * LC], ps)

            # ---- phase 2: kT[q, l] and v[m, q] ----
            for c in range(NLC):
                for q in range(NPT):
                    ps = psS.tile([128, LC], F32, tag="psS")
                    for p in range(NPT):
                        nc.tensor.matmul(
                            ps,
                            w2b[p][:, q * 128 : (q + 1) * 128],
                            ztb[p][:, c * LC : (c + 1) * LC],
                            start=(p == 0),
                            stop=(p == NPT - 1),
                        )
                    nc.vector.tensor_copy(kt[q][:, c * LC : (c + 1) * LC], ps)
            for t in range(NMT):
                ps = psS.tile([128, P], F32, tag="psS")
                for p in range(NPT):
                    nc.tensor.matmul(
                        ps,
                        ztb[p][:, t * 128 : (t + 1) * 128],
                        w2b[p][:, P:],
                        start=(p == 0),
                        stop=(p == NPT - 1),
                    )
                nc.vector.tensor_copy(vt[t], ps)

            # ---- phases 3-5 per l-chunk: scoresT, exp, sums, outT, res, y ----
            # The y-matmuls for chunk c are interleaved into chunk c+1's
            # j-loop, and outT accumulation lags the scores pipeline by
            # OUT_DELAY iterations, so the PE never stalls on the DVE
            # normalize chain (recip -> broadcast -> res) at chunk edges.
            OUT_DELAY = 2

            def emit_y_group(res, cprev, t, oc):
                lt = cprev * 4 + t
                psy = psS.tile([128, LC], F32, tag="psS", name="psy")
                for p in range(NPT):
                    nc.tensor.matmul(
                        psy,
                        r(res[p][:, t * 128 : (t + 1) * 128]),
                        r(w3[p][:, oc * LC : (oc + 1) * LC]),
                        start=(p == 0),
                        stop=(p == NPT - 1),
                    )
                ysb = work.tile([128, LC], F32, tag="xt_4", name="ysb")
                nc.vector.tensor_copy(ysb, psy)
                nc.sync.dma_start(
                    y[lt * 128 : (lt + 1) * 128, oc * LC : (oc + 1) * LC], ysb
                )

            def lparams(c, j):
                s = j - 4 * c  # >=0 on diagonal m-tiles
                return max(s, 0) * 128, s  # first valid l within chunk

            prev_res = None  # res tiles of previous chunk (feeds its y)
            for c in range(NLC):
                njt = 4 * c + 4  # valid m-tiles for this chunk (m <= l)
                yq = (
                    [(prev_res, c - 1, t, oc) for t in range(4) for oc in range(2)]
                    if prev_res is not None
                    else []
                )
                pss = psSum.tile([1, LC], F32, tag="psSum", name="pss")
                pso = [psO.tile([128, LC], F32, tag=f"psO_{p}", name=f"psO_{p}") for p in range(NPT)]
                ets = {}

                def emit_out(c, j):
                    loff, _ = lparams(c, j)
                    et = ets.pop(j)
                    nc.tensor.matmul(
                        pss[:, loff:],
                        ones,
                        et[:, loff:],
                        start=(j == 0),
                        stop=(j == njt - 1),
                    )
                    for p in range(NPT):
                        nc.tensor.matmul(
                            pso[p][:, loff:],
                            vt[j][:, p * 128 : (p + 1) * 128],
                            et[:, loff:],
                            start=(j == 0),
                            stop=(j == njt - 1),
                        )

                for j in range(njt):
                    loff, s = lparams(c, j)
                    ps = psS.tile([128, LC], F32, tag="psS", name="ps")
                    for p in range(NPT):
                        nc.tensor.matmul(
                            ps[:, loff:],
                            kt[p][:, j * 128 : (j + 1) * 128],
                            ztb[p][:, c * LC + loff : (c + 1) * LC],
                            start=(p == 0),
                            stop=(p == NPT - 1),
                        )
                    et = etp.tile([128, LC], BF16, tag="et", name="et")
                    # exp(scale * scores); scale folds the q-scaling in
                    nc.scalar.activation(
                        et[:, loff:],
                        ps[:, loff:],
                        mybir.ActivationFunctionType.Exp,
                        scale=SCALE,
                    )
                    if s >= 0:
                        # zero the upper-left triangle of the diagonal block
                        nc.vector.tensor_mul(
                            et[:, loff : loff + 128],
                            et[:, loff : loff + 128],
                            tri,
                        )
                    ets[j] = et
                    if yq:
                        emit_y_group(*yq.pop(0))
                    if j >= OUT_DELAY:
                        emit_out(c, j - OUT_DELAY)
                for g in yq:
                    emit_y_group(*g)
                for j in range(max(njt - OUT_DELAY, 0), njt):
                    emit_out(c, j)
                # normalize + residual: resT = oT * (1/sums) + zT
                rT = work.tile([1, LC], F32, tag="rT")
                nc.vector.reciprocal_approx_fast(out=rT, in_=pss)
                # reuse phase-1 xt slots (same shape) to stay in SBUF budget
                rb = work.tile([128, LC], F32, tag="xt_5")
                nc.gpsimd.partition_broadcast(rb, rT)
                res = [work.tile([128, LC], F32, tag=f"xt_{p}", name=f"res_{p}") for p in range(NPT)]
                for p in range(NPT):
                    nc.vector.tensor_mul(r(res[p]), pso[p], rb)
                    nc.vector.tensor_add(
                        r(res[p]), res[p], zt[p][:, c * LC : (c + 1) * LC]
                    )
                prev_res = res
            for t in range(4):
                for oc in range(2):
                    emit_y_group(prev_res, NLC - 1, t, oc)
    nc.compile()
    return nc


def _get_nc():
    if "nc" not in _CACHE:
        _CACHE["nc"] = _build()
    return _CACHE["nc"]


def kernel(x, W1, W2, W3):
    from concourse.bass_utils import run_bass_kernel_spmd

    x = np.asarray(x, dtype=np.float32)
    W1 = np.asarray(W1, dtype=np.float32)
    W2 = np.asarray(W2, dtype=np.float32)
    W3 = np.asarray(W3, dtype=np.float32)

    w1t = np.ascontiguousarray(W1.T)  # [D, P]
    w2t = np.ascontiguousarray(W2.T)  # [P, 2P]
    w3t = np.ascontiguousarray(W3.T)  # [P, DO]

    nc = _get_nc()
    in_maps = [
        {
            "xT": np.ascontiguousarray(x[:, b, :].T),
            "w1t": w1t,
            "w2t": w2t,
            "w3t": w3t,
        }
        for b in range(B)
    ]
    res = run_bass_kernel_spmd(nc, in_maps, core_ids=list(range(B)))
    _CACHE["last_result"] = res
    return np.stack([res.results[b]["y"] for b in range(B)], axis=1)


# revision 14
# speedup vs baseline: 1.1482x; 1.1482x over previous
"""Trainium2 Bass kernel for nn_Custom_U_2_88630945120527.

Dense transformer block:
    z = x @ W1.T                    # [L, B, P]
    kv = z @ W2.T ; k, v = split    # [L, B, 2P]
    scores = (z*scale) @ k.T        # per-batch [L, L], causal
    attn = softmax(scores)
    out = attn @ v                  # [L, B, P]
    y = (out + z) @ W3.T            # [L, B, D_OUT]

Sharding: data-parallel over batch (B=8 -> 8 cores, one batch element per
core, weights replicated, no collectives).

Per-core layout strategy: the host pre-transposes x (xT = x[:,b,:].T) and
the weights (W1T/W2T/W3T), so every matmul in the chain has its
contraction dim on partitions with zero on-chip transposes:
    zT[p,l]   = sum_d W1T[d,p] * xT[d,l]
    kT[q,l]   = sum_p W2T[p,q] * zT[p,l]          (q in [0,P))
    v[m,q]    = sum_p zT[p,m]  * W2T[p,P+q]
    sT[m,l]   = sum_p kT[p,m]  * zT[p,l]          (computed transposed!)
    eT[m,l]   = exp(scale*sT) with causal mask    (ScalarE, scale fused)
    sums[1,l] = sum_m ones[m] * eT[m,l]           (matmul w/ ones lhsT)
    oT[p,l]   = sum_m v[m,p]  * eT[m,l]
    resT[p,l] = oT[p,l] / sums[l] + zT[p,l]
    y[l,o]    = sum_p resT[p,l] * W3T[p,o]
Computing scores TRANSPOSED puts m (the softmax-reduction axis) on
partitions so attention weights feed the out-matmul as lhsT directly.
Softmax skips the max-subtraction (scores are O(5), exp is safe in fp32;
the reference's masked entries underflow to exactly 0, matched by zeroing).

All matmuls run as float32r (full PE rate, ~1.5e-4 rel err measured on HW
vs fp32) with fp32 PSUM accumulation.
"""

import numpy as np

L, B, D, P, DO = 2048, 8, 1024, 512, 1024
LC = 512  # l-chunk width (PSUM bank / fp32)
NLC = L // LC  # 4 chunks
NPT = P // 128  # 4 p-tiles
NDT = D // 128  # 8 d-tiles
NMT = L // 128  # 16 m-tiles
SCALE = float(P) ** -0.5

_CACHE = {}


def _build():
    import concourse.bacc as bacc
    import concourse.mybir as mybir
    import concourse.tile as tile
    from concourse.masks import make_upper_triangular

    F32 = mybir.dt.float32
    F32R = mybir.dt.float32r
    BF16 = mybir.dt.bfloat16

    def r(ap):
        return ap.bitcast(F32R)

    nc = bacc.Bacc("TRN2", debug=False)
    xT = nc.dram_tensor("xT", [D, L], F32R, kind="ExternalInput")
    w1t = nc.dram_tensor("w1t", [D, P], F32R, kind="ExternalInput")
    w2t = nc.dram_tensor("w2t", [P, 2 * P], F32R, kind="ExternalInput")
    w3t = nc.dram_tensor("w3t", [P, DO], F32R, kind="ExternalInput")
    y = nc.dram_tensor("y", [L, DO], F32, kind="ExternalOutput")

    with tile.TileContext(nc) as tc:
        with (
            tc.tile_pool(name="consts", bufs=1) as consts,
            tc.tile_pool(name="weights", bufs=1) as wpool,
            tc.tile_pool(name="persist", bufs=1) as persist,
            tc.tile_pool(name="work", bufs=2) as work,
            tc.tile_pool(name="etp", bufs=6) as etp,
            tc.tile_pool(name="psS", bufs=3, space="PSUM") as psS,
            tc.tile_pool(name="psO", bufs=1, space="PSUM") as psO,
            tc.tile_pool(name="psSum", bufs=1, space="PSUM") as psSum,
        ):
            # ---- constants ----
            zcol = consts.tile([128, 1], F32, tag="zcol")
            nc.vector.memset(zcol, 0.0)
            ones = consts.tile([128, 1], F32, tag="ones")
            nc.vector.tensor_scalar_add(r(ones), zcol, 1.0)
            # tri[m, l] = 1.0 where l >= m else 0 (within a 128x128 block)
            tri = consts.tile([128, 128], F32, tag="tri")
            make_upper_triangular(nc, tri, val=1.0, diag=True)

            # ---- weights (pre-transposed on host) ----
            w1 = [wpool.tile([128, P], F32, tag=f"w1_{d}", name=f"w1_{d}") for d in range(NDT)]
            for d in range(NDT):
                nc.sync.dma_start(r(w1[d]), w1t[d * 128 : (d + 1) * 128, :])
            w2 = [wpool.tile([128, 2 * P], F32, tag=f"w2_{p}", name=f"w2_{p}") for p in range(NPT)]
            for p in range(NPT):
                nc.sync.dma_start(r(w2[p]), w2t[p * 128 : (p + 1) * 128, :])
            w3 = [wpool.tile([128, DO], F32, tag=f"w3_{p}", name=f"w3_{p}") for p in range(NPT)]
            for p in range(NPT):
                nc.sync.dma_start(r(w3[p]), w3t[p * 128 : (p + 1) * 128, :])

            zt = [persist.tile([128, L], F32, tag=f"zt_{p}", name=f"zt_{p}") for p in range(NPT)]
            kt = [persist.tile([128, L], F32, tag=f"kt_{p}", name=f"kt_{p}") for p in range(NPT)]
            vt = [persist.tile([128, P], F32, tag=f"v_{t}", name=f"v_{t}") for t in range(NMT)]

            # ---- phase 1: zT[p, l] ----
            for c in range(NLC):
                xtile = [work.tile([128, LC], F32, tag=f"xt_{d}", name=f"xt_{d}") for d in range(NDT)]
                for d in range(NDT):
                    nc.sync.dma_start(
                        r(xtile[d]),
                        xT[d * 128 : (d + 1) * 128, c * LC : (c + 1) * LC],
                    )
                for p in range(NPT):
                    ps = psS.tile([128, LC], F32, tag="psS")
                    for d in range(NDT):
                        nc.tensor.matmul(
                            ps,
                            r(w1[d][:, p * 128 : (p + 1) * 128]),
                            r(xtile[d]),
                            start=(d == 0),
                            stop=(d == NDT - 1),
                        )
                    nc.vector.tensor_copy(r(zt[p][:, c * LC : (c + 1) * LC]), ps)

            # ---- phase 2: kT[q, l] and v[m, q] ----
            for c in range(NLC):
                for q in range(NPT):
                    ps = psS.tile([128, LC], F32, tag="psS")
                    for p in range(NPT):
                        nc.tensor.matmul(
                            ps,
                            r(w2[p][:, q * 128 : (q + 1) * 128]),
                            r(zt[p][:, c * LC : (c + 1) * LC]),
                            start=(p == 0),
                            stop=(p == NPT - 1),
                        )
                    nc.vector.tensor_copy(r(kt[q][:, c * LC : (c + 1) * LC]), ps)
            for t in range(NMT):
                ps = psS.tile([128, P], F32, tag="psS")
                for p in range(NPT):
                    nc.tensor.matmul(
                        ps,
                        r(zt[p][:, t * 128 : (t + 1) * 128]),
                        r(w2[p][:, P:]),
                        start=(p == 0),
                        stop=(p == NPT - 1),
                    )
                nc.vector.tensor_copy(r(vt[t]), ps)

            # ---- phases 3-5 per l-chunk: scoresT, exp, sums, outT, res, y ----
            # The y-matmuls for chunk c are interleaved into chunk c+1's
            # j-loop, and outT accumulation lags the scores pipeline by
            # OUT_DELAY iterations, so the PE never stalls on the DVE
            # normalize chain (recip -> broadcast -> res) at chunk edges.
            OUT_DELAY = 2

            def emit_y_group(res, cprev, t, oc):
                lt = cprev * 4 + t
                psy = psS.tile([128, LC], F32, tag="psS", name="psy")
                for p in range(NPT):
                    nc.tensor.matmul(
                        psy,
                        r(res[p][:, t * 128 : (t + 1) * 128]),
                        r(w3[p][:, oc * LC : (oc + 1) * LC]),
                        start=(p == 0),
                        stop=(p == NPT - 1),
                    )
                ysb = work.tile([128, LC], F32, tag="xt_4", name="ysb")
                nc.vector.tensor_copy(ysb, psy)
                nc.sync.dma_start(
                    y[lt * 128 : (lt + 1) * 128, oc * LC : (oc + 1) * LC], ysb
                )

            def lparams(c, j):
                s = j - 4 * c  # >=0 on diagonal m-tiles
                return max(s, 0) * 128, s  # first valid l within chunk

            prev_res = None  # res tiles of previous chunk (feeds its y)
            for c in range(NLC):
                njt = 4 * c + 4  # valid m-tiles for this chunk (m <= l)
                yq = (
                    [(prev_res, c - 1, t, oc) for t in range(4) for oc in range(2)]
                    if prev_res is not None
                    else []
                )
                pss = psSum.tile([1, LC], F32, tag="psSum", name="pss")
                pso = [psO.tile([128, LC], F32, tag=f"psO_{p}", name=f"psO_{p}") for p in range(NPT)]
                ets = {}

                def emit_out(c, j):
                    loff, _ = lparams(c, j)
                    et = ets.pop(j)
                    nc.tensor.matmul(
                        pss[:, loff:],
                        r(ones),
                        r(et[:, loff:]),
                        start=(j == 0),
                        stop=(j == njt - 1),
                    )
                    for p in range(NPT):
                        nc.tensor.matmul(
                            pso[p][:, loff:],
                            r(vt[j][:, p * 128 : (p + 1) * 128]),
                            r(et[:, loff:]),
                            start=(j == 0),
                            stop=(j == njt - 1),
                        )

                for j in range(njt):
                    loff, s = lparams(c, j)
                    ps = psS.tile([128, LC], F32, tag="psS", name="ps")
                    for p in range(NPT):
                        nc.tensor.matmul(
                            ps[:, loff:],
                            r(kt[p][:, j * 128 : (j + 1) * 128]),
                            r(zt[p][:, c * LC + loff : (c + 1) * LC]),
                            start=(p == 0),
                            stop=(p == NPT - 1),
                        )
                    et = etp.tile([128, LC], F32, tag="et", name="et")
                    # exp(scale * scores); scale folds the q-scaling in
                    nc.scalar.activation(
                        r(et[:, loff:]),
                        ps[:, loff:],
                        mybir.ActivationFunctionType.Exp,
                        scale=SCALE,
                    )
                    if s >= 0:
                        # zero the upper-left triangle of the diagonal block
                        nc.vector.tensor_mul(
                            r(et[:, loff : loff + 128]),
                            et[:, loff : loff + 128],
                            tri,
                        )
                    ets[j] = et
                    if yq:
                        emit_y_group(*yq.pop(0))
                    if j >= OUT_DELAY:
                        emit_out(c, j - OUT_DELAY)
                for g in yq:
                    emit_y_group(*g)
                for j in range(max(njt - OUT_DELAY, 0), njt):
                    emit_out(c, j)
                # normalize + residual: resT = oT * (1/sums) + zT
                rT = work.tile([1, LC], F32, tag="rT")
                nc.vector.reciprocal_approx_fast(out=rT, in_=pss)
                # reuse phase-1 xt slots (same shape) to stay in SBUF budget
                rb = work.tile([128, LC], F32, tag="xt_5")
                nc.gpsimd.partition_broadcast(rb, rT)
                res = [work.tile([128, LC], F32, tag=f"xt_{p}", name=f"res_{p}") for p in range(NPT)]
                for p in range(NPT):
                    nc.vector.tensor_mul(r(res[p]), pso[p], rb)
                    nc.vector.tensor_add(
                        r(res[p]), res[p], zt[p][:, c * LC : (c + 1) * LC]
                    )
                prev_res = res
            for t in range(4):
                for oc in range(2):
                    emit_y_group(prev_res, NLC - 1, t, oc)
    nc.compile()
    return nc


def _get_nc():
    if "nc" not in _CACHE:
        _CACHE["nc"] = _build()
    return _CACHE["nc"]


def kernel(x, W1, W2, W3):
    from concourse.bass_utils import run_bass_kernel_spmd

    x = np.asarray(x, dtype=np.float32)
    W1 = np.asarray(W1, dtype=np.float32)
    W2 = np.asarray(W2, dtype=np.float32)
    W3 = np.asarray(W3, dtype=np.float32)

    w1t = np.ascontiguousarray(W1.T)  # [D, P]
    w2t = np.ascontiguousarray(W2.T)  # [P, 2P]
    w3t = np.ascontiguousarray(W3.T)  # [P, DO]

    nc = _get_nc()
    in_maps = [
        {
            "xT": np.ascontiguousarray(x[:, b, :].T),
            "w1t": w1t,
            "w2t": w2t,
            "w3t": w3t,
        }
        for b in range(B)
    ]
    res = run_bass_kernel_spmd(nc, in_maps, core_ids=list(range(B)))
    _CACHE["last_result"] = res
    return np.stack([res.results[b]["y"] for b in range(B)], axis=1)


# revision 15
# speedup vs baseline: 1.1776x; 1.0256x over previous
"""Trainium2 Bass kernel for nn_Custom_U_2_88630945120527.

Dense transformer block:
    z = x @ W1.T                    # [L, B, P]
    kv = z @ W2.T ; k, v = split    # [L, B, 2P]
    scores = (z*scale) @ k.T        # per-batch [L, L], causal
    attn = softmax(scores)
    out = attn @ v                  # [L, B, P]
    y = (out + z) @ W3.T            # [L, B, D_OUT]

Sharding: data-parallel over batch (B=8 -> 8 cores, one batch element per
core, weights replicated, no collectives).

Per-core layout strategy: the host pre-transposes x (xT = x[:,b,:].T) and
the weights (W1T/W2T/W3T), so every matmul in the chain has its
contraction dim on partitions with zero on-chip transposes:
    zT[p,l]   = sum_d W1T[d,p] * xT[d,l]
    kT[q,l]   = sum_p W2T[p,q] * zT[p,l]          (q in [0,P))
    v[m,q]    = sum_p zT[p,m]  * W2T[p,P+q]
    sT[m,l]   = sum_p kT[p,m]  * zT[p,l]          (computed transposed!)
    eT[m,l]   = exp(scale*sT) with causal mask    (ScalarE, scale fused)
    sums[1,l] = sum_m ones[m] * eT[m,l]           (matmul w/ ones lhsT)
    oT[p,l]   = sum_m v[m,p]  * eT[m,l]
    resT[p,l] = oT[p,l] / sums[l] + zT[p,l]
    y[l,o]    = sum_p resT[p,l] * W3T[p,o]
Computing scores TRANSPOSED puts m (the softmax-reduction axis) on
partitions so attention weights feed the out-matmul as lhsT directly.
Softmax skips the max-subtraction (scores are O(5), exp is safe in fp32;
the reference's masked entries underflow to exactly 0, matched by zeroing).

All matmuls run as float32r (full PE rate, ~1.5e-4 rel err measured on HW
vs fp32) with fp32 PSUM accumulation.
"""

import numpy as np

L, B, D, P, DO = 2048, 8, 1024, 512, 1024
LC = 512  # l-chunk width (PSUM bank / fp32)
NLC = L // LC  # 4 chunks
NPT = P // 128  # 4 p-tiles
NDT = D // 128  # 8 d-tiles
NMT = L // 128  # 16 m-tiles
SCALE = float(P) ** -0.5

_CACHE = {}


def _build():
    import concourse.bacc as bacc
    import concourse.mybir as mybir
    import concourse.tile as tile
    from concourse.masks import make_upper_triangular

    F32 = mybir.dt.float32
    F32R = mybir.dt.float32r
    BF16 = mybir.dt.bfloat16

    def r(ap):
        return ap.bitcast(F32R)

    nc = bacc.Bacc("TRN2", debug=False)
    xT = nc.dram_tensor("xT", [D, L], F32R, kind="ExternalInput")
    w1t = nc.dram_tensor("w1t", [D, P], F32R, kind="ExternalInput")
    w2t = nc.dram_tensor("w2t", [P, 2 * P], F32R, kind="ExternalInput")
    w3t = nc.dram_tensor("w3t", [P, DO], F32R, kind="ExternalInput")
    y = nc.dram_tensor("y", [L, DO], F32, kind="ExternalOutput")

    with tile.TileContext(nc) as tc:
        with (
            tc.tile_pool(name="consts", bufs=1) as consts,
            tc.tile_pool(name="weights", bufs=1) as wpool,
            tc.tile_pool(name="persist", bufs=1) as persist,
            tc.tile_pool(name="work", bufs=2) as work,
            tc.tile_pool(name="etp", bufs=6) as etp,
            tc.tile_pool(name="psS", bufs=3, space="PSUM") as psS,
            tc.tile_pool(name="psO", bufs=1, space="PSUM") as psO,
            tc.tile_pool(name="psSum", bufs=1, space="PSUM") as psSum,
        ):
            # ---- constants ----
            zcol = consts.tile([128, 1], F32, tag="zcol")
            nc.vector.memset(zcol, 0.0)
            ones = consts.tile([128, 1], F32, tag="ones")
            nc.vector.tensor_scalar_add(r(ones), zcol, 1.0)
            # tri[m, l] = 1.0 where l >= m else 0 (within a 128x128 block)
            tri = consts.tile([128, 128], F32, tag="tri")
            make_upper_triangular(nc, tri, val=1.0, diag=True)

            # ---- weights (pre-transposed on host) ----
            w1 = [wpool.tile([128, P], F32, tag=f"w1_{d}", name=f"w1_{d}") for d in range(NDT)]
            w2 = [wpool.tile([128, 2 * P], F32, tag=f"w2_{p}", name=f"w2_{p}") for p in range(NPT)]
            w3 = [wpool.tile([128, DO], F32, tag=f"w3_{p}", name=f"w3_{p}") for p in range(NPT)]

            zt = [persist.tile([128, L], F32, tag=f"zt_{p}", name=f"zt_{p}") for p in range(NPT)]
            kt = [persist.tile([128, L], F32, tag=f"kt_{p}", name=f"kt_{p}") for p in range(NPT)]
            vt = [persist.tile([128, P], F32, tag=f"v_{t}", name=f"v_{t}") for t in range(NMT)]

            # ---- phase 1: zT[p, l] ----
            # DMA order matters: the first matmul needs w1[d] + xtile[d] pairs,
            # so interleave them; w2/w3 (needed a phase later) load behind.
            for c in range(NLC):
                xtile = [work.tile([128, LC], F32, tag=f"xt_{d}", name=f"xt_{d}") for d in range(NDT)]
                for d in range(NDT):
                    if c == 0:
                        nc.sync.dma_start(r(w1[d]), w1t[d * 128 : (d + 1) * 128, :])
                    nc.sync.dma_start(
                        r(xtile[d]),
                        xT[d * 128 : (d + 1) * 128, c * LC : (c + 1) * LC],
                    )
                if c == 1:
                    for p in range(NPT):
                        nc.sync.dma_start(r(w2[p]), w2t[p * 128 : (p + 1) * 128, :])
                if c == 2:
                    for p in range(NPT):
                        nc.sync.dma_start(r(w3[p]), w3t[p * 128 : (p + 1) * 128, :])
                for p in range(NPT):
                    ps = psS.tile([128, LC], F32, tag="psS")
                    for d in range(NDT):
                        nc.tensor.matmul(
                            ps,
                            r(w1[d][:, p * 128 : (p + 1) * 128]),
                            r(xtile[d]),
                            start=(d == 0),
                            stop=(d == NDT - 1),
                        )
                    nc.vector.tensor_copy(r(zt[p][:, c * LC : (c + 1) * LC]), ps)

            # ---- phase 2: kT[q, l] and v[m, q] ----
            for c in range(NLC):
                for q in range(NPT):
                    ps = psS.tile([128, LC], F32, tag="psS")
                    for p in range(NPT):
                        nc.tensor.matmul(
                            ps,
                            r(w2[p][:, q * 128 : (q + 1) * 128]),
                            r(zt[p][:, c * LC : (c + 1) * LC]),
                            start=(p == 0),
                            stop=(p == NPT - 1),
                        )
                    nc.vector.tensor_copy(r(kt[q][:, c * LC : (c + 1) * LC]), ps)
            for t in range(NMT):
                ps = psS.tile([128, P], F32, tag="psS")
                for p in range(NPT):
                    nc.tensor.matmul(
                        ps,
                        r(zt[p][:, t * 128 : (t + 1) * 128]),
                        r(w2[p][:, P:]),
                        start=(p == 0),
                        stop=(p == NPT - 1),
                    )
                nc.vector.tensor_copy(r(vt[t]), ps)

            # ---- phases 3-5 per l-chunk: scoresT, exp, sums, outT, res, y ----
            # The y-matmuls for chunk c are interleaved into chunk c+1's
            # j-loop, and outT accumulation lags the scores pipeline by
            # OUT_DELAY iterations, so the PE never stalls on the DVE
            # normalize chain (recip -> broadcast -> res) at chunk edges.
            OUT_DELAY = 3

            def emit_y_group(res, cprev, t, oc):
                lt = cprev * 4 + t
                psy = psS.tile([128, LC], F32, tag="psS", name="psy")
                for p in range(NPT):
                    nc.tensor.matmul(
                        psy,
                        r(res[p][:, t * 128 : (t + 1) * 128]),
                        r(w3[p][:, oc * LC : (oc + 1) * LC]),
                        start=(p == 0),
                        stop=(p == NPT - 1),
                    )
                ysb = work.tile([128, LC], F32, tag="xt_4", name="ysb")
                nc.vector.tensor_copy(ysb, psy)
                nc.sync.dma_start(
                    y[lt * 128 : (lt + 1) * 128, oc * LC : (oc + 1) * LC], ysb
                )

            def lparams(c, j):
                s = j - 4 * c  # >=0 on diagonal m-tiles
                return max(s, 0) * 128, s  # first valid l within chunk

            prev_res = None  # res tiles of previous chunk (feeds its y)
            for c in range(NLC):
                njt = 4 * c + 4  # valid m-tiles for this chunk (m <= l)
                yq = (
                    [(prev_res, c - 1, t, oc) for t in range(4) for oc in range(2)]
                    if prev_res is not None
                    else []
                )
                pss = psSum.tile([1, LC], F32, tag="psSum", name="pss")
                pso = [psO.tile([128, LC], F32, tag=f"psO_{p}", name=f"psO_{p}") for p in range(NPT)]
                ets = {}

                def emit_out(c, j):
                    loff, _ = lparams(c, j)
                    et = ets.pop(j)
                    nc.tensor.matmul(
                        pss[:, loff:],
                        r(ones),
                        r(et[:, loff:]),
                        start=(j == 0),
                        stop=(j == njt - 1),
                    )
                    for p in range(NPT):
                        nc.tensor.matmul(
                            pso[p][:, loff:],
                            r(vt[j][:, p * 128 : (p + 1) * 128]),
                            r(et[:, loff:]),
                            start=(j == 0),
                            stop=(j == njt - 1),
                        )

                for j in range(njt):
                    loff, s = lparams(c, j)
                    ps = psS.tile([128, LC], F32, tag="psS", name="ps")
                    for p in range(NPT):
                        nc.tensor.matmul(
                            ps[:, loff:],
                            r(kt[p][:, j * 128 : (j + 1) * 128]),
                            r(zt[p][:, c * LC + loff : (c + 1) * LC]),
                            start=(p == 0),
                            stop=(p == NPT - 1),
                        )
                    et = etp.tile([128, LC], F32, tag="et", name="et")
                    # exp(scale * scores); scale folds the q-scaling in
                    nc.scalar.activation(
                        r(et[:, loff:]),
                        ps[:, loff:],
                        mybir.ActivationFunctionType.Exp,
                        scale=SCALE,
                    )
                    if s >= 0:
                        # zero the upper-left triangle of the diagonal block
                        nc.vector.tensor_mul(
                            r(et[:, loff : loff + 128]),
                            et[:, loff : loff + 128],
                            tri,
                        )
                    ets[j] = et
                    if yq:
                        emit_y_group(*yq.pop(0))
                    if j >= OUT_DELAY:
                        emit_out(c, j - OUT_DELAY)
                for g in yq:
                    emit_y_group(*g)
                for j in range(max(njt - OUT_DELAY, 0), njt):
                    emit_out(c, j)
                # normalize + residual: resT = oT * (1/sums) + zT
                rT = work.tile([1, LC], F32, tag="rT")
                nc.vector.reciprocal_approx_fast(out=rT, in_=pss)
                # reuse phase-1 xt slots (same shape) to stay in SBUF budget
                rb = work.tile([128, LC], F32, tag="xt_5")
                nc.gpsimd.partition_broadcast(rb, rT)
                res = [work.tile([128, LC], F32, tag=f"xt_{p}", name=f"res_{p}") for p in range(NPT)]
                for p in range(NPT):
                    nc.vector.tensor_mul(r(res[p]), pso[p], rb)
                    nc.vector.tensor_add(
                        r(res[p]), res[p], zt[p][:, c * LC : (c + 1) * LC]
                    )
                prev_res = res
            for t in range(4):
                for oc in range(2):
                    emit_y_group(prev_res, NLC - 1, t, oc)
    nc.compile()
    return nc


def _get_nc():
    if "nc" not in _CACHE:
        _CACHE["nc"] = _build()
    return _CACHE["nc"]


def kernel(x, W1, W2, W3):
    from concourse.bass_utils import run_bass_kernel_spmd

    x = np.asarray(x, dtype=np.float32)
    W1 = np.asarray(W1, dtype=np.float32)
    W2 = np.asarray(W2, dtype=np.float32)
    W3 = np.asarray(W3, dtype=np.float32)

    w1t = np.ascontiguousarray(W1.T)  # [D, P]
    w2t = np.ascontiguousarray(W2.T)  # [P, 2P]
    w3t = np.ascontiguousarray(W3.T)  # [P, DO]

    nc = _get_nc()
    in_maps = [
        {
            "xT": np.ascontiguousarray(x[:, b, :].T),
            "w1t": w1t,
            "w2t": w2t,
            "w3t": w3t,
        }
        for b in range(B)
    ]
    res = run_bass_kernel_spmd(nc, in_maps, core_ids=list(range(B)))
    _CACHE["last_result"] = res
    return np.stack([res.results[b]["y"] for b in range(B)], axis=1)


# revision 16
# speedup vs baseline: 1.1857x; 1.0069x over previous
"""Trainium2 Bass kernel for nn_Custom_U_2_88630945120527.

Dense transformer block:
    z = x @ W1.T                    # [L, B, P]
    kv = z @ W2.T ; k, v = split    # [L, B, 2P]
    scores = (z*scale) @ k.T        # per-batch [L, L], causal
    attn = softmax(scores)
    out = attn @ v                  # [L, B, P]
    y = (out + z) @ W3.T            # [L, B, D_OUT]

Sharding: data-parallel over batch (B=8 -> 8 cores, one batch element per
core, weights replicated, no collectives).

Per-core layout strategy: the host pre-transposes x (xT = x[:,b,:].T) and
the weights (W1T/W2T/W3T), so every matmul in the chain has its
contraction dim on partitions with zero on-chip transposes:
    zT[p,l]   = sum_d W1T[d,p] * xT[d,l]
    kT[q,l]   = sum_p W2T[p,q] * zT[p,l]          (q in [0,P))
    v[m,q]    = sum_p zT[p,m]  * W2T[p,P+q]
    sT[m,l]   = sum_p kT[p,m]  * zT[p,l]          (computed transposed!)
    eT[m,l]   = exp(scale*sT) with causal mask    (ScalarE, scale fused)
    sums[1,l] = sum_m ones[m] * eT[m,l]           (matmul w/ ones lhsT)
    oT[p,l]   = sum_m v[m,p]  * eT[m,l]
    resT[p,l] = oT[p,l] / sums[l] + zT[p,l]
    y[l,o]    = sum_p resT[p,l] * W3T[p,o]
Computing scores TRANSPOSED puts m (the softmax-reduction axis) on
partitions so attention weights feed the out-matmul as lhsT directly.
Softmax skips the max-subtraction (scores are O(5), exp is safe in fp32;
the reference's masked entries underflow to exactly 0, matched by zeroing).

All matmuls run as float32r (full PE rate, ~1.5e-4 rel err measured on HW
vs fp32) with fp32 PSUM accumulation.
"""

import numpy as np

L, B, D, P, DO = 2048, 8, 1024, 512, 1024
LC = 512  # l-chunk width (PSUM bank / fp32)
NLC = L // LC  # 4 chunks
NPT = P // 128  # 4 p-tiles
NDT = D // 128  # 8 d-tiles
NMT = L // 128  # 16 m-tiles
SCALE = float(P) ** -0.5

_CACHE = {}


def _build():
    import concourse.bacc as bacc
    import concourse.mybir as mybir
    import concourse.tile as tile
    from concourse.masks import make_upper_triangular

    F32 = mybir.dt.float32
    F32R = mybir.dt.float32r
    BF16 = mybir.dt.bfloat16

    def r(ap):
        return ap.bitcast(F32R)

    nc = bacc.Bacc("TRN2", debug=False)
    xT = nc.dram_tensor("xT", [D, L], F32R, kind="ExternalInput")
    w1t = nc.dram_tensor("w1t", [D, P], F32R, kind="ExternalInput")
    w2t = nc.dram_tensor("w2t", [P, 2 * P], F32R, kind="ExternalInput")
    w3t = nc.dram_tensor("w3t", [P, DO], F32R, kind="ExternalInput")
    y = nc.dram_tensor("y", [L, DO], F32, kind="ExternalOutput")

    with tile.TileContext(nc) as tc:
        with (
            tc.tile_pool(name="consts", bufs=1) as consts,
            tc.tile_pool(name="weights", bufs=1) as wpool,
            tc.tile_pool(name="persist", bufs=1) as persist,
            tc.tile_pool(name="work", bufs=2) as work,
            tc.tile_pool(name="etp", bufs=7) as etp,
            tc.tile_pool(name="psS", bufs=3, space="PSUM") as psS,
            tc.tile_pool(name="psO", bufs=1, space="PSUM") as psO,
            tc.tile_pool(name="psSum", bufs=1, space="PSUM") as psSum,
        ):
            # ---- constants ----
            zcol = consts.tile([128, 1], F32, tag="zcol")
            nc.vector.memset(zcol, 0.0)
            ones = consts.tile([128, 1], F32, tag="ones")
            nc.vector.tensor_scalar_add(r(ones), zcol, 1.0)
            # warm the ScalarE Exp table before attention needs it
            warmup = consts.tile([128, 1], F32, tag="warmup")
            nc.scalar.activation(
                warmup, zcol, mybir.ActivationFunctionType.Exp, scale=1.0
            )
            # tri[m, l] = 1.0 where l >= m else 0 (within a 128x128 block)
            tri = consts.tile([128, 128], F32, tag="tri")
            make_upper_triangular(nc, tri, val=1.0, diag=True)

            # ---- weights (pre-transposed on host) ----
            w1 = [wpool.tile([128, P], F32, tag=f"w1_{d}", name=f"w1_{d}") for d in range(NDT)]
            w2 = [wpool.tile([128, 2 * P], F32, tag=f"w2_{p}", name=f"w2_{p}") for p in range(NPT)]
            w3 = [wpool.tile([128, DO], F32, tag=f"w3_{p}", name=f"w3_{p}") for p in range(NPT)]

            zt = [persist.tile([128, L], F32, tag=f"zt_{p}", name=f"zt_{p}") for p in range(NPT)]
            kt = [persist.tile([128, L], F32, tag=f"kt_{p}", name=f"kt_{p}") for p in range(NPT)]
            vt = [persist.tile([128, P], F32, tag=f"v_{t}", name=f"v_{t}") for t in range(NMT)]

            # ---- phase 1: zT[p, l] ----
            # DMA order matters: the first matmul needs w1[d] + xtile[d] pairs,
            # so interleave them; w2/w3 (needed a phase later) load behind.
            for c in range(NLC):
                xtile = [work.tile([128, LC], F32, tag=f"xt_{d}", name=f"xt_{d}") for d in range(NDT)]
                for d in range(NDT):
                    if c == 0:
                        nc.sync.dma_start(r(w1[d]), w1t[d * 128 : (d + 1) * 128, :])
                    nc.sync.dma_start(
                        r(xtile[d]),
                        xT[d * 128 : (d + 1) * 128, c * LC : (c + 1) * LC],
                    )
                if c == NLC - 1:
                    for p in range(NPT):
                        nc.sync.dma_start(r(w2[p]), w2t[p * 128 : (p + 1) * 128, :])
                    for p in range(NPT):
                        nc.sync.dma_start(r(w3[p]), w3t[p * 128 : (p + 1) * 128, :])
                for p in range(NPT):
                    ps = psS.tile([128, LC], F32, tag="psS")
                    for d in range(NDT):
                        nc.tensor.matmul(
                            ps,
                            r(w1[d][:, p * 128 : (p + 1) * 128]),
                            r(xtile[d]),
                            start=(d == 0),
                            stop=(d == NDT - 1),
                        )
                    nc.vector.tensor_copy(r(zt[p][:, c * LC : (c + 1) * LC]), ps)

            # ---- phase 2: kT[q, l] and v[m, q] ----
            for c in range(NLC):
                for q in range(NPT):
                    ps = psS.tile([128, LC], F32, tag="psS")
                    for p in range(NPT):
                        nc.tensor.matmul(
                            ps,
                            r(w2[p][:, q * 128 : (q + 1) * 128]),
                            r(zt[p][:, c * LC : (c + 1) * LC]),
                            start=(p == 0),
                            stop=(p == NPT - 1),
                        )
                    nc.vector.tensor_copy(r(kt[q][:, c * LC : (c + 1) * LC]), ps)
            for t in range(NMT):
                ps = psS.tile([128, P], F32, tag="psS")
                for p in range(NPT):
                    nc.tensor.matmul(
                        ps,
                        r(zt[p][:, t * 128 : (t + 1) * 128]),
                        r(w2[p][:, P:]),
                        start=(p == 0),
                        stop=(p == NPT - 1),
                    )
                nc.vector.tensor_copy(r(vt[t]), ps)

            # ---- phases 3-5 per l-chunk: scoresT, exp, sums, outT, res, y ----
            # The y-matmuls for chunk c are interleaved into chunk c+1's
            # j-loop, and outT accumulation lags the scores pipeline by
            # OUT_DELAY iterations, so the PE never stalls on the DVE
            # normalize chain (recip -> broadcast -> res) at chunk edges.
            OUT_DELAY = 4

            def emit_y_group(res, cprev, t, oc):
                lt = cprev * 4 + t
                psy = psS.tile([128, LC], F32, tag="psS", name="psy")
                for p in range(NPT):
                    nc.tensor.matmul(
                        psy,
                        r(res[p][:, t * 128 : (t + 1) * 128]),
                        r(w3[p][:, oc * LC : (oc + 1) * LC]),
                        start=(p == 0),
                        stop=(p == NPT - 1),
                    )
                ysb = work.tile([128, LC], F32, tag="xt_4", name="ysb")
                nc.vector.tensor_copy(ysb, psy)
                nc.sync.dma_start(
                    y[lt * 128 : (lt + 1) * 128, oc * LC : (oc + 1) * LC], ysb
                )

            def lparams(c, j):
                s = j - 4 * c  # >=0 on diagonal m-tiles
                return max(s, 0) * 128, s  # first valid l within chunk

            prev_res = None  # res tiles of previous chunk (feeds its y)
            for c in range(NLC):
                njt = 4 * c + 4  # valid m-tiles for this chunk (m <= l)
                yq = (
                    [(prev_res, c - 1, t, oc) for t in range(4) for oc in range(2)]
                    if prev_res is not None
                    else []
                )
                pss = psSum.tile([1, LC], F32, tag="psSum", name="pss")
                pso = [psO.tile([128, LC], F32, tag=f"psO_{p}", name=f"psO_{p}") for p in range(NPT)]
                ets = {}

                def emit_out(c, j):
                    loff, _ = lparams(c, j)
                    et = ets.pop(j)
                    nc.tensor.matmul(
                        pss[:, loff:],
                        r(ones),
                        r(et[:, loff:]),
                        start=(j == 0),
                        stop=(j == njt - 1),
                    )
                    for p in range(NPT):
                        nc.tensor.matmul(
                            pso[p][:, loff:],
                            r(vt[j][:, p * 128 : (p + 1) * 128]),
                            r(et[:, loff:]),
                            start=(j == 0),
                            stop=(j == njt - 1),
                        )

                for j in range(njt):
                    loff, s = lparams(c, j)
                    ps = psS.tile([128, LC], F32, tag="psS", name="ps")
                    for p in range(NPT):
                        nc.tensor.matmul(
                            ps[:, loff:],
                            r(kt[p][:, j * 128 : (j + 1) * 128]),
                            r(zt[p][:, c * LC + loff : (c + 1) * LC]),
                            start=(p == 0),
                            stop=(p == NPT - 1),
                        )
                    et = etp.tile([128, LC], F32, tag="et", name="et")
                    # exp(scale * scores); scale folds the q-scaling in
                    nc.scalar.activation(
                        r(et[:, loff:]),
                        ps[:, loff:],
                        mybir.ActivationFunctionType.Exp,
                        scale=SCALE,
                    )
                    if s >= 0:
                        # zero the upper-left triangle of the diagonal block
                        nc.vector.tensor_mul(
                            r(et[:, loff : loff + 128]),
                            et[:, loff : loff + 128],
                            tri,
                        )
                    ets[j] = et
                    if yq:
                        emit_y_group(*yq.pop(0))
                    if j >= OUT_DELAY:
                        emit_out(c, j - OUT_DELAY)
                for g in yq:
                    emit_y_group(*g)
                for j in range(max(njt - OUT_DELAY, 0), njt):
                    emit_out(c, j)
                # normalize + residual: resT = oT * (1/sums) + zT
                rT = work.tile([1, LC], F32, tag="rT")
                nc.vector.reciprocal_approx_fast(out=rT, in_=pss)
                # reuse phase-1 xt slots (same shape) to stay in SBUF budget
                rb = work.tile([128, LC], F32, tag="xt_5")
                nc.gpsimd.partition_broadcast(rb, rT)
                res = [work.tile([128, LC], F32, tag=f"xt_{p}", name=f"res_{p}") for p in range(NPT)]
                for p in range(NPT):
                    nc.vector.tensor_mul(r(res[p]), pso[p], rb)
                    nc.vector.tensor_add(
                        r(res[p]), res[p], zt[p][:, c * LC : (c + 1) * LC]
                    )
                prev_res = res
            for t in range(4):
                for oc in range(2):
                    emit_y_group(prev_res, NLC - 1, t, oc)
    nc.compile()
    return nc


def _get_nc():
    if "nc" not in _CACHE:
        _CACHE["nc"] = _build()
    return _CACHE["nc"]


def kernel(x, W1, W2, W3):
    from concourse.bass_utils import run_bass_kernel_spmd

    x = np.asarray(x, dtype=np.float32)
    W1 = np.asarray(W1, dtype=np.float32)
    W2 = np.asarray(W2, dtype=np.float32)
    W3 = np.asarray(W3, dtype=np.float32)

    w1t = np.ascontiguousarray(W1.T)  # [D, P]
    w2t = np.ascontiguousarray(W2.T)  # [P, 2P]
    w3t = np.ascontiguousarray(W3.T)  # [P, DO]

    nc = _get_nc()
    in_maps = [
        {
            "xT": np.ascontiguousarray(x[:, b, :].T),
            "w1t": w1t,
            "w2t": w2t,
            "w3t": w3t,
        }
        for b in range(B)
    ]
    res = run_bass_kernel_spmd(nc, in_maps, core_ids=list(range(B)))
    _CACHE["last_result"] = res
    return np.stack([res.results[b]["y"] for b in range(B)], axis=1)


# revision 17
# speedup vs baseline: 1.2406x; 1.0463x over previous
"""Trainium2 Bass kernel for nn_Custom_U_2_88630945120527.

Dense transformer block:
    z = x @ W1.T                    # [L, B, P]
    kv = z @ W2.T ; k, v = split    # [L, B, 2P]
    scores = (z*scale) @ k.T        # per-batch [L, L], causal
    attn = softmax(scores)
    out = attn @ v                  # [L, B, P]
    y = (out + z) @ W3.T            # [L, B, D_OUT]

Sharding: data-parallel over batch (B=8 -> 8 cores, one batch element per
core, weights replicated, no collectives).

Per-core layout strategy: the host pre-transposes x (xT = x[:,b,:].T) and
the weights (W1T/W2T/W3T), so every matmul in the chain has its
contraction dim on partitions with zero on-chip transposes:
    zT[p,l]   = sum_d W1T[d,p] * xT[d,l]
    kT[q,l]   = sum_p W2T[p,q] * zT[p,l]          (q in [0,P))
    v[m,q]    = sum_p zT[p,m]  * W2T[p,P+q]
    sT[m,l]   = sum_p kT[p,m]  * zT[p,l]          (computed transposed!)
    eT[m,l]   = exp(scale*sT) with causal mask    (ScalarE, scale fused)
    sums[1,l] = sum_m ones[m] * eT[m,l]           (matmul w/ ones lhsT)
    oT[p,l]   = sum_m v[m,p]  * eT[m,l]
    resT[p,l] = oT[p,l] / sums[l] + zT[p,l]
    y[l,o]    = sum_p resT[p,l] * W3T[p,o]
Computing scores TRANSPOSED puts m (the softmax-reduction axis) on
partitions so attention weights feed the out-matmul as lhsT directly.
Softmax skips the max-subtraction (scores are O(5), exp is safe in fp32;
the reference's masked entries underflow to exactly 0, matched by zeroing).

All matmuls run as float32r (full PE rate, ~1.5e-4 rel err measured on HW
vs fp32) with fp32 PSUM accumulation.
"""

import numpy as np

L, B, D, P, DO = 2048, 8, 1024, 512, 1024
LC = 512  # l-chunk width (PSUM bank / fp32)
NLC = L // LC  # 4 chunks
NPT = P // 128  # 4 p-tiles
NDT = D // 128  # 8 d-tiles
NMT = L // 128  # 16 m-tiles
SCALE = float(P) ** -0.5

_CACHE = {}


def _build():
    import concourse.bacc as bacc
    import concourse.mybir as mybir
    import concourse.tile as tile
    from concourse.masks import make_upper_triangular

    F32 = mybir.dt.float32
    F32R = mybir.dt.float32r
    BF16 = mybir.dt.bfloat16

    def r(ap):
        return ap.bitcast(F32R)

    nc = bacc.Bacc("TRN2", debug=False)
    xT = nc.dram_tensor("xT", [D, L], F32R, kind="ExternalInput")
    w1t = nc.dram_tensor("w1t", [D, P], F32R, kind="ExternalInput")
    w2t = nc.dram_tensor("w2t", [P, 2 * P], F32R, kind="ExternalInput")
    w3t = nc.dram_tensor("w3t", [P, DO], F32R, kind="ExternalInput")
    y = nc.dram_tensor("y", [L, DO], F32, kind="ExternalOutput")

    with tile.TileContext(nc) as tc:
        with (
            tc.tile_pool(name="consts", bufs=1) as consts,
            tc.tile_pool(name="weights", bufs=1) as wpool,
            tc.tile_pool(name="persist", bufs=1) as persist,
            tc.tile_pool(name="work", bufs=2) as work,
            tc.tile_pool(name="etp", bufs=9) as etp,
            tc.tile_pool(name="psS", bufs=3, space="PSUM") as psS,
            tc.tile_pool(name="psO", bufs=1, space="PSUM") as psO,
            tc.tile_pool(name="psSum", bufs=1, space="PSUM") as psSum,
        ):
            # ---- constants ----
            zcol = consts.tile([128, 1], F32, tag="zcol")
            nc.vector.memset(zcol, 0.0)
            ones = consts.tile([128, 1], F32, tag="ones")
            nc.vector.tensor_scalar_add(r(ones), zcol, 1.0)
            # warm the ScalarE Exp table before attention needs it
            warmup = consts.tile([128, 1], F32, tag="warmup")
            nc.scalar.activation(
                warmup, zcol, mybir.ActivationFunctionType.Exp, scale=1.0
            )
            # tri[m, l] = 1.0 where l >= m else 0 (within a 128x128 block)
            tri = consts.tile([128, 128], F32, tag="tri")
            make_upper_triangular(nc, tri, val=1.0, diag=True)

            # ---- weights (pre-transposed on host) ----
            w1 = [wpool.tile([128, P], F32, tag=f"w1_{d}", name=f"w1_{d}") for d in range(NDT)]
            w2 = [wpool.tile([128, 2 * P], F32, tag=f"w2_{p}", name=f"w2_{p}") for p in range(NPT)]
            w3 = [wpool.tile([128, DO], F32, tag=f"w3_{p}", name=f"w3_{p}") for p in range(NPT)]

            zt = [persist.tile([128, L], F32, tag=f"zt_{p}", name=f"zt_{p}") for p in range(NPT)]
            kt = [persist.tile([128, L], F32, tag=f"kt_{p}", name=f"kt_{p}") for p in range(NPT)]
            vt = [persist.tile([128, P], F32, tag=f"v_{t}", name=f"v_{t}") for t in range(NMT)]

            # ---- phase 1: zT[p, l] ----
            # DMA order matters: the first matmul needs w1[d] + xtile[d] pairs,
            # so interleave them; w2/w3 (needed a phase later) load behind.
            for c in range(NLC):
                xtile = [work.tile([128, LC], F32, tag=f"xt_{d}", name=f"xt_{d}") for d in range(NDT)]
                for d in range(NDT):
                    if c == 0:
                        nc.sync.dma_start(r(w1[d]), w1t[d * 128 : (d + 1) * 128, :])
                    nc.sync.dma_start(
                        r(xtile[d]),
                        xT[d * 128 : (d + 1) * 128, c * LC : (c + 1) * LC],
                    )
                if c == NLC - 1:
                    for p in range(NPT):
                        nc.sync.dma_start(r(w2[p]), w2t[p * 128 : (p + 1) * 128, :])
                    for p in range(NPT):
                        nc.sync.dma_start(r(w3[p]), w3t[p * 128 : (p + 1) * 128, :])
                for p in range(NPT):
                    ps = psS.tile([128, LC], F32, tag="psS")
                    for d in range(NDT):
                        nc.tensor.matmul(
                            ps,
                            r(w1[d][:, p * 128 : (p + 1) * 128]),
                            r(xtile[d]),
                            start=(d == 0),
                            stop=(d == NDT - 1),
                        )
                    nc.vector.tensor_copy(r(zt[p][:, c * LC : (c + 1) * LC]), ps)

            # ---- phase 2: kT[q, l] and v[m, q] ----
            for c in range(NLC):
                for q in range(NPT):
                    ps = psS.tile([128, LC], F32, tag="psS")
                    for p in range(NPT):
                        nc.tensor.matmul(
                            ps,
                            r(w2[p][:, q * 128 : (q + 1) * 128]),
                            r(zt[p][:, c * LC : (c + 1) * LC]),
                            start=(p == 0),
                            stop=(p == NPT - 1),
                        )
                    nc.vector.tensor_copy(r(kt[q][:, c * LC : (c + 1) * LC]), ps)
            for t in range(NMT):
                ps = psS.tile([128, P], F32, tag="psS")
                for p in range(NPT):
                    nc.tensor.matmul(
                        ps,
                        r(zt[p][:, t * 128 : (t + 1) * 128]),
                        r(w2[p][:, P:]),
                        start=(p == 0),
                        stop=(p == NPT - 1),
                    )
                nc.vector.tensor_copy(r(vt[t]), ps)

            # ---- phases 3-5 per l-chunk: scoresT, exp, sums, outT, res, y ----
            # The y-matmuls for chunk c are interleaved into chunk c+1's
            # j-loop, and outT accumulation lags the scores pipeline by
            # OUT_DELAY iterations, so the PE never stalls on the DVE
            # normalize chain (recip -> broadcast -> res) at chunk edges.
            OUT_DELAY = 7

            def emit_y_group(res, cprev, t, oc):
                lt = cprev * 4 + t
                psy = psS.tile([128, LC], F32, tag="psS", name="psy")
                for p in range(NPT):
                    nc.tensor.matmul(
                        psy,
                        r(res[p][:, t * 128 : (t + 1) * 128]),
                        r(w3[p][:, oc * LC : (oc + 1) * LC]),
                        start=(p == 0),
                        stop=(p == NPT - 1),
                    )
                ysb = work.tile([128, LC], F32, tag="xt_4", name="ysb")
                nc.vector.tensor_copy(ysb, psy)
                nc.sync.dma_start(
                    y[lt * 128 : (lt + 1) * 128, oc * LC : (oc + 1) * LC], ysb
                )

            def lparams(c, j):
                s = j - 4 * c  # >=0 on diagonal m-tiles
                return max(s, 0) * 128, s  # first valid l within chunk

            prev_res = None  # res tiles of previous chunk (feeds its y)
            for c in range(NLC):
                njt = 4 * c + 4  # valid m-tiles for this chunk (m <= l)
                yq = (
                    [(prev_res, c - 1, t, oc) for t in range(4) for oc in range(2)]
                    if prev_res is not None
                    else []
                )
                pss = psSum.tile([1, LC], F32, tag="psSum", name="pss")
                pso = [psO.tile([128, LC], F32, tag=f"psO_{p}", name=f"psO_{p}") for p in range(NPT)]
                ets = {}

                def emit_out(c, j):
                    loff, _ = lparams(c, j)
                    et = ets.pop(j)
                    nc.tensor.matmul(
                        pss[:, loff:],
                        r(ones),
                        r(et[:, loff:]),
                        start=(j == 0),
                        stop=(j == njt - 1),
                    )
                    for p in range(NPT):
                        nc.tensor.matmul(
                            pso[p][:, loff:],
                            r(vt[j][:, p * 128 : (p + 1) * 128]),
                            r(et[:, loff:]),
                            start=(j == 0),
                            stop=(j == njt - 1),
                        )

                for j in range(njt):
                    loff, s = lparams(c, j)
                    ps = psS.tile([128, LC], F32, tag="psS", name="ps")
                    for p in range(NPT):
                        nc.tensor.matmul(
                            ps[:, loff:],
                            r(kt[p][:, j * 128 : (j + 1) * 128]),
                            r(zt[p][:, c * LC + loff : (c + 1) * LC]),
                            start=(p == 0),
                            stop=(p == NPT - 1),
                        )
                    et = etp.tile([128, LC], F32, tag="et", name="et")
                    # exp(scale * scores); scale folds the q-scaling in
                    nc.scalar.activation(
                        r(et[:, loff:]),
                        ps[:, loff:],
                        mybir.ActivationFunctionType.Exp,
                        scale=SCALE,
                    )
                    if s >= 0:
                        # zero the upper-left triangle of the diagonal block
                        nc.vector.tensor_mul(
                            r(et[:, loff : loff + 128]),
                            et[:, loff : loff + 128],
                            tri,
                        )
                    ets[j] = et
                    if j >= OUT_DELAY:
                        for _ in range(2):
                            if yq:
                                emit_y_group(*yq.pop(0))
                        emit_out(c, j - OUT_DELAY)
                for g in yq:
                    emit_y_group(*g)
                for j in range(max(njt - OUT_DELAY, 0), njt):
                    emit_out(c, j)
                # normalize + residual: resT = oT * (1/sums) + zT
                rT = work.tile([1, LC], F32, tag="rT")
                nc.vector.reciprocal_approx_fast(out=rT, in_=pss)
                # reuse phase-1 xt slots (same shape) to stay in SBUF budget
                rb = work.tile([128, LC], F32, tag="xt_5")
                nc.gpsimd.partition_broadcast(rb, rT)
                res = [work.tile([128, LC], F32, tag=f"xt_{p}", name=f"res_{p}") for p in range(NPT)]
                for p in range(NPT):
                    nc.vector.tensor_mul(r(res[p]), pso[p], rb)
                    nc.vector.tensor_add(
                        r(res[p]), res[p], zt[p][:, c * LC : (c + 1) * LC]
                    )
                prev_res = res
            for t in range(4):
                for oc in range(2):
                    emit_y_group(prev_res, NLC - 1, t, oc)
    nc.compile()
    return nc


def _get_nc():
    if "nc" not in _CACHE:
        _CACHE["nc"] = _build()
    return _CACHE["nc"]


def kernel(x, W1, W2, W3):
    from concourse.bass_utils import run_bass_kernel_spmd

    x = np.asarray(x, dtype=np.float32)
    W1 = np.asarray(W1, dtype=np.float32)
    W2 = np.asarray(W2, dtype=np.float32)
    W3 = np.asarray(W3, dtype=np.float32)

    w1t = np.ascontiguousarray(W1.T)  # [D, P]
    w2t = np.ascontiguousarray(W2.T)  # [P, 2P]
    w3t = np.ascontiguousarray(W3.T)  # [P, DO]

    nc = _get_nc()
    in_maps = [
        {
            "xT": np.ascontiguousarray(x[:, b, :].T),
            "w1t": w1t,
            "w2t": w2t,
            "w3t": w3t,
        }
        for b in range(B)
    ]
    res = run_bass_kernel_spmd(nc, in_maps, core_ids=list(range(B)))
    _CACHE["last_result"] = res
    return np.stack([res.results[b]["y"] for b in range(B)], axis=1)


# revision 18
# speedup vs baseline: 1.2885x; 1.0386x over previous
"""Trainium2 Bass kernel for nn_Custom_U_2_88630945120527.

Dense transformer block:
    z = x @ W1.T                    # [L, B, P]
    kv = z @ W2.T ; k, v = split    # [L, B, 2P]
    scores = (z*scale) @ k.T        # per-batch [L, L], causal
    attn = softmax(scores)
    out = attn @ v                  # [L, B, P]
    y = (out + z) @ W3.T            # [L, B, D_OUT]

Sharding: data-parallel over batch (B=8 -> 8 cores, one batch element per
core, weights replicated, no collectives).

Per-core layout strategy: the host pre-transposes x (xT = x[:,b,:].T) and
the weights (W1T/W2T/W3T), so every matmul in the chain has its
contraction dim on partitions with zero on-chip transposes:
    zT[p,l]   = sum_d W1T[d,p] * xT[d,l]
    kT[q,l]   = sum_p W2T[p,q] * zT[p,l]          (q in [0,P))
    v[m,q]    = sum_p zT[p,m]  * W2T[p,P+q]
    sT[m,l]   = sum_p kT[p,m]  * zT[p,l]          (computed transposed!)
    eT[m,l]   = exp(scale*sT) with causal mask    (ScalarE, scale fused)
    sums[1,l] = sum_m ones[m] * eT[m,l]           (matmul w/ ones lhsT)
    oT[p,l]   = sum_m v[m,p]  * eT[m,l]
    resT[p,l] = oT[p,l] / sums[l] + zT[p,l]
    y[l,o]    = sum_p resT[p,l] * W3T[p,o]
Computing scores TRANSPOSED puts m (the softmax-reduction axis) on
partitions so attention weights feed the out-matmul as lhsT directly.
Softmax skips the max-subtraction (scores are O(5), exp is safe in fp32;
the reference's masked entries underflow to exactly 0, matched by zeroing).

All matmuls run as float32r (full PE rate, ~1.5e-4 rel err measured on HW
vs fp32) with fp32 PSUM accumulation.
"""

import numpy as np

L, B, D, P, DO = 2048, 8, 1024, 512, 1024
LC = 512  # l-chunk width (PSUM bank / fp32)
NLC = L // LC  # 4 chunks
NPT = P // 128  # 4 p-tiles
NDT = D // 128  # 8 d-tiles
NMT = L // 128  # 16 m-tiles
SCALE = float(P) ** -0.5

_CACHE = {}


def _build():
    import concourse.bacc as bacc
    import concourse.mybir as mybir
    import concourse.tile as tile
    from concourse.masks import make_upper_triangular

    F32 = mybir.dt.float32
    F32R = mybir.dt.float32r
    BF16 = mybir.dt.bfloat16

    def r(ap):
        return ap.bitcast(F32R)

    nc = bacc.Bacc("TRN2", debug=False)
    xT = nc.dram_tensor("xT", [D, L], F32R, kind="ExternalInput")
    w1t = nc.dram_tensor("w1t", [D, P], F32R, kind="ExternalInput")
    w2t = nc.dram_tensor("w2t", [P, 2 * P], F32R, kind="ExternalInput")
    w3t = nc.dram_tensor("w3t", [P, DO], F32R, kind="ExternalInput")
    y = nc.dram_tensor("y", [L, DO], F32, kind="ExternalOutput")

    with tile.TileContext(nc) as tc:
        with (
            tc.tile_pool(name="consts", bufs=1) as consts,
            tc.tile_pool(name="weights", bufs=1) as wpool,
            tc.tile_pool(name="persist", bufs=1) as persist,
            tc.tile_pool(name="work", bufs=2) as work,
            tc.tile_pool(name="etp", bufs=9) as etp,
            tc.tile_pool(name="psS", bufs=3, space="PSUM") as psS,
            tc.tile_pool(name="psO", bufs=1, space="PSUM") as psO,
            tc.tile_pool(name="psSum", bufs=1, space="PSUM") as psSum,
        ):
            # ---- constants ----
            zcol = consts.tile([128, 1], F32, tag="zcol")
            nc.vector.memset(zcol, 0.0)
            ones = consts.tile([128, 1], F32, tag="ones")
            nc.vector.tensor_scalar_add(r(ones), zcol, 1.0)
            zrow = consts.tile([1, 128], F32, tag="zrow")
            nc.vector.memset(zrow, 0.0)
            ones_row = consts.tile([1, 128], F32, tag="ones_row")
            nc.vector.tensor_scalar_add(r(ones_row), zrow, 1.0)
            # warm the ScalarE Exp table before attention needs it
            warmup = consts.tile([128, 1], F32, tag="warmup")
            nc.scalar.activation(
                warmup, zcol, mybir.ActivationFunctionType.Exp, scale=1.0
            )
            # tri[m, l] = 1.0 where l >= m else 0 (within a 128x128 block)
            tri = consts.tile([128, 128], F32, tag="tri")
            make_upper_triangular(nc, tri, val=1.0, diag=True)

            # ---- weights (pre-transposed on host) ----
            w1 = [wpool.tile([128, P], F32, tag=f"w1_{d}", name=f"w1_{d}") for d in range(NDT)]
            w2 = [wpool.tile([128, 2 * P], F32, tag=f"w2_{p}", name=f"w2_{p}") for p in range(NPT)]
            w3 = [wpool.tile([128, DO], F32, tag=f"w3_{p}", name=f"w3_{p}") for p in range(NPT)]

            zt = [persist.tile([128, L], F32, tag=f"zt_{p}", name=f"zt_{p}") for p in range(NPT)]
            kt = [persist.tile([128, L], F32, tag=f"kt_{p}", name=f"kt_{p}") for p in range(NPT)]
            vt = [persist.tile([128, P], F32, tag=f"v_{t}", name=f"v_{t}") for t in range(NMT)]

            # ---- phase 1: zT[p, l] ----
            # DMA order matters: the first matmul needs w1[d] + xtile[d] pairs,
            # so interleave them; w2/w3 (needed a phase later) load behind.
            for c in range(NLC):
                xtile = [work.tile([128, LC], F32, tag=f"xt_{d}", name=f"xt_{d}") for d in range(NDT)]
                for d in range(NDT):
                    if c == 0:
                        nc.sync.dma_start(r(w1[d]), w1t[d * 128 : (d + 1) * 128, :])
                    nc.sync.dma_start(
                        r(xtile[d]),
                        xT[d * 128 : (d + 1) * 128, c * LC : (c + 1) * LC],
                    )
                if c == NLC - 1:
                    for p in range(NPT):
                        nc.sync.dma_start(r(w2[p]), w2t[p * 128 : (p + 1) * 128, :])
                    for p in range(NPT):
                        nc.sync.dma_start(r(w3[p]), w3t[p * 128 : (p + 1) * 128, :])
                for p in range(NPT):
                    ps = psS.tile([128, LC], F32, tag="psS")
                    for d in range(NDT):
                        nc.tensor.matmul(
                            ps,
                            r(w1[d][:, p * 128 : (p + 1) * 128]),
                            r(xtile[d]),
                            start=(d == 0),
                            stop=(d == NDT - 1),
                        )
                    nc.vector.tensor_copy(r(zt[p][:, c * LC : (c + 1) * LC]), ps)

            # ---- phase 2: kT[q, l] and v[m, q] ----
            for c in range(NLC):
                for q in range(NPT):
                    ps = psS.tile([128, LC], F32, tag="psS")
                    for p in range(NPT):
                        nc.tensor.matmul(
                            ps,
                            r(w2[p][:, q * 128 : (q + 1) * 128]),
                            r(zt[p][:, c * LC : (c + 1) * LC]),
                            start=(p == 0),
                            stop=(p == NPT - 1),
                        )
                    nc.vector.tensor_copy(r(kt[q][:, c * LC : (c + 1) * LC]), ps)
            for t in range(NMT):
                ps = psS.tile([128, P], F32, tag="psS")
                for p in range(NPT):
                    nc.tensor.matmul(
                        ps,
                        r(zt[p][:, t * 128 : (t + 1) * 128]),
                        r(w2[p][:, P:]),
                        start=(p == 0),
                        stop=(p == NPT - 1),
                    )
                nc.vector.tensor_copy(r(vt[t]), ps)

            # ---- phases 3-5 per l-chunk: scoresT, exp, sums, outT, res, y ----
            # The y-matmuls for chunk c are interleaved into chunk c+1's
            # j-loop, and outT accumulation lags the scores pipeline by
            # OUT_DELAY iterations, so the PE never stalls on the DVE
            # normalize chain (recip -> broadcast -> res) at chunk edges.
            OUT_DELAY = 7

            def emit_y_group(res, cprev, t, oc):
                lt = cprev * 4 + t
                psy = psS.tile([128, LC], F32, tag="psS", name="psy")
                for p in range(NPT):
                    nc.tensor.matmul(
                        psy,
                        r(res[p][:, t * 128 : (t + 1) * 128]),
                        r(w3[p][:, oc * LC : (oc + 1) * LC]),
                        start=(p == 0),
                        stop=(p == NPT - 1),
                    )
                ysb = work.tile([128, LC], F32, tag="xt_4", name="ysb")
                nc.vector.tensor_copy(ysb, psy)
                nc.sync.dma_start(
                    y[lt * 128 : (lt + 1) * 128, oc * LC : (oc + 1) * LC], ysb
                )

            def lparams(c, j):
                s = j - 4 * c  # >=0 on diagonal m-tiles
                return max(s, 0) * 128, s  # first valid l within chunk

            prev_res = None  # res tiles of previous chunk (feeds its y)
            for c in range(NLC):
                njt = 4 * c + 4  # valid m-tiles for this chunk (m <= l)
                yq = (
                    [(prev_res, c - 1, t, oc) for t in range(4) for oc in range(2)]
                    if prev_res is not None
                    else []
                )
                pss = psSum.tile([1, LC], F32, tag="psSum", name="pss")
                pso = [psO.tile([128, LC], F32, tag=f"psO_{p}", name=f"psO_{p}") for p in range(NPT)]
                ets = {}

                def emit_out(c, j):
                    loff, _ = lparams(c, j)
                    et = ets.pop(j)
                    nc.tensor.matmul(
                        pss[:, loff:],
                        r(ones),
                        r(et[:, loff:]),
                        start=(j == 0),
                        stop=(j == njt - 1),
                    )
                    for p in range(NPT):
                        nc.tensor.matmul(
                            pso[p][:, loff:],
                            r(vt[j][:, p * 128 : (p + 1) * 128]),
                            r(et[:, loff:]),
                            start=(j == 0),
                            stop=(j == njt - 1),
                        )

                for j in range(njt):
                    loff, s = lparams(c, j)
                    ps = psS.tile([128, LC], F32, tag="psS", name="ps")
                    for p in range(NPT):
                        nc.tensor.matmul(
                            ps[:, loff:],
                            r(kt[p][:, j * 128 : (j + 1) * 128]),
                            r(zt[p][:, c * LC + loff : (c + 1) * LC]),
                            start=(p == 0),
                            stop=(p == NPT - 1),
                        )
                    et = etp.tile([128, LC], F32, tag="et", name="et")
                    # exp(scale * scores); scale folds the q-scaling in
                    nc.scalar.activation(
                        r(et[:, loff:]),
                        ps[:, loff:],
                        mybir.ActivationFunctionType.Exp,
                        scale=SCALE,
                    )
                    if s >= 0:
                        # zero the upper-left triangle of the diagonal block
                        nc.vector.tensor_mul(
                            r(et[:, loff : loff + 128]),
                            et[:, loff : loff + 128],
                            tri,
                        )
                    ets[j] = et
                    if j >= OUT_DELAY:
                        for _ in range(2):
                            if yq:
                                emit_y_group(*yq.pop(0))
                        emit_out(c, j - OUT_DELAY)
                for g in yq:
                    emit_y_group(*g)
                for j in range(max(njt - OUT_DELAY, 0), njt):
                    emit_out(c, j)
                # normalize + residual: resT = oT * (1/sums) + zT.
                # Broadcast sums across partitions with a K=1 PE matmul
                # (ones_row^T @ sums_row) -- GPSIMD partition_broadcast costs
                # a library swap + long sem waits; the PE does it in ~0.2us.
                srow = work.tile([1, LC], F32, tag="rT")
                nc.vector.tensor_copy(r(srow), pss)
                rbp = psS.tile([128, LC], F32, tag="psS", name="rbp")
                nc.tensor.matmul(rbp, r(ones_row), r(srow), start=True, stop=True)
                # reuse phase-1 xt slots (same shape) to stay in SBUF budget
                rb = work.tile([128, LC], F32, tag="xt_5")
                nc.vector.reciprocal_approx_fast(out=rb, in_=rbp)
                res = [work.tile([128, LC], F32, tag=f"xt_{p}", name=f"res_{p}") for p in range(NPT)]
                for p in range(NPT):
                    nc.vector.tensor_mul(r(res[p]), pso[p], rb)
                    nc.vector.tensor_add(
                        r(res[p]), res[p], zt[p][:, c * LC : (c + 1) * LC]
                    )
                prev_res = res
            for t in range(4):
                for oc in range(2):
                    emit_y_group(prev_res, NLC - 1, t, oc)
    nc.compile()
    return nc


def _get_nc():
    if "nc" not in _CACHE:
        _CACHE["nc"] = _build()
    return _CACHE["nc"]


def kernel(x, W1, W2, W3):
    from concourse.bass_utils import run_bass_kernel_spmd

    x = np.asarray(x, dtype=np.float32)
    W1 = np.asarray(W1, dtype=np.float32)
    W2 = np.asarray(W2, dtype=np.float32)
    W3 = np.asarray(W3, dtype=np.float32)

    w1t = np.ascontiguousarray(W1.T)  # [D, P]
    w2t = np.ascontiguousarray(W2.T)  # [P, 2P]
    w3t = np.ascontiguousarray(W3.T)  # [P, DO]

    nc = _get_nc()
    in_maps = [
        {
            "xT": np.ascontiguousarray(x[:, b, :].T),
            "w1t": w1t,
            "w2t": w2t,
            "w3t": w3t,
        }
        for b in range(B)
    ]
    res = run_bass_kernel_spmd(nc, in_maps, core_ids=list(range(B)))
    _CACHE["last_result"] = res
    return np.stack([res.results[b]["y"] for b in range(B)], axis=1)
